# revision 11
# baseline (speedup 1.0000x reference)
"""Distributed Trainium2 kernel for nn_ARLoss_88390426951926.

Computes mean(loss) where, per element (EPS = 1e-6):
    c = round(t); d = x - c; pos = d >= 0
    z = pos ? ceil(x) : floor(x)
    loss = max(0, |d| - |x - z| + pos*EPS)

Algebraic reduction (validated ~5e-6 rel err on the real data):
    With F = floor(x), S = 2x - c - F, S5 = S - 0.5:
        loss = relu(S - 1 + eps) + relu(-S)         (ties measure-zero)
so  sum(loss) = sum relu(S5 - .5) + sum relu(-S5 - .5)
             = sum max(S5, .5) - N/2 + sum relu(-S5 - .5).

Rounding via the float32 magic trick (M = 1.5*2^23), fused to minimize
DVE passes (reverse0 on tensor_scalar + scalar_tensor_tensor both
HW-verified exact by probe.py):
    y2 = (0.5 - x) + M     = M - floor(x)        DVE TS2op reverse0, f32
    e  = (t + M) - y2      = round(t) + floor(x) DVE STT, f32 chain->bf16
    x2h = Copy(2x - 0.5)                         ACT, f32->bf16
    S5 = x2h - e                                 DVE TT, bf16
    qp = max(S5, 0.5); PE psum[1,512] += ones.T @ qp
    ACT relu-accum: acc2[:,col] = sum relu(-S5 - 0.5)   (all segments)
The relu-accum for segment k is emitted during segment k+1 so ACT's
x2h pass runs one segment ahead and never stalls the DVE chain.
Per-core budget (measured): DVE ~72us, ACT ~72us, PE ~45us under the
~94-102us HBM DMA floor (33.5 MB @ ~330-358 GB/s).
First and last tiles are split into 4 quarter tiles so the pipeline
ramps in and drains out quickly.
"""

import sys
import types

import numpy as np

import concourse.bass as bass
import concourse.bacc as bacc
import concourse.mybir as mybir
from concourse.tile import TileContext
from concourse.bass_utils import run_bass_kernel_spmd


def _ensure_axon_hooks():
    """Some agent images lack ``antenv.axon_hooks``; if BASS_TRACE is set
    in the environment, run_bass_kernel_spmd imports it and would crash.
    Provide a no-op hook registry so tracing degrades gracefully."""
    try:
        import antenv  # noqa: F401
    except ImportError:
        return
    try:
        import antenv.axon_hooks  # noqa: F401
        return
    except ImportError:
        pass
    mod = types.ModuleType("antenv.axon_hooks")
    _state = {"hook": None}
    mod.set_axon_ntff_profile_hook = lambda h: _state.__setitem__("hook", h)
    mod.get_axon_ntff_profile_hook = lambda: _state["hook"]
    sys.modules["antenv.axon_hooks"] = mod
    import antenv as _a

    _a.axon_hooks = mod


_ensure_axon_hooks()

B, D = 8192, 4096
N_CORES = 8
ROWS = B // N_CORES          # 1024 rows per core
P = 128                      # SBUF partitions
FD = 4096                    # free dim per full tile
NTILES = (ROWS * D) // (P * FD)   # 8 full tiles per core
MAGIC = 12582912.0           # 1.5 * 2**23
MM_N = 512                   # matmul free-dim chunk (one PSUM bank)

F32 = mybir.dt.float32
BF16 = mybir.dt.bfloat16

HFD = FD // 2                # compute half-tile width (smaller mid pool)

# Per-tile DMA split: first/last tiles in graduated sub-transfers (fast
# ramp/drain) but into ONE SBUF slot each, so the splits don't burn
# extra tile-pool slots; middle tiles as full 2 MiB transfers.
TILE_DMAS = []
for _ti in range(NTILES):
    if _ti == 0:
        TILE_DMAS.append([(0, 512), (512, 512), (1024, 1024), (2048, 2048)])
    elif _ti == NTILES - 1:
        TILE_DMAS.append([(0, 2048), (2048, 1024), (3072, 512), (3584, 512)])
    else:
        TILE_DMAS.append([(0, FD)])

# Compute segments: (tile_idx, tile_off, fd, typeB). Edge tiles are
# computed in segments aligned with their sub-DMAs; full tiles in two
# half-tiles so mid intermediates are [P, HFD]. typeB segments compute
# the plus-branch as 1/2*(PE sum(S5) + ACT sum|S5-.5|) instead of the
# DVE max pass, moving ~690ns/half from the DVE (the binding engine)
# to ACT+PE slack. Alternating halves in the middle tiles gives an
# element fraction ~0.375, near the DVE==ACT balance point.
COMP_SEGS = []
_hidx = 0
for _ti, _subs in enumerate(TILE_DMAS):
    for _off, _dfd in _subs:
        for _h in range(max(1, _dfd // HFD)):
            _fd = min(HFD, _dfd)
            _tb = False
            if _fd == HFD and 0 < _ti < NTILES - 1:
                _tb = _hidx % 2 == 1
                _hidx += 1
            COMP_SEGS.append((_ti, _off + _h * HFD, _fd, _tb))
N_COLS = len(COMP_SEGS)
N_A = sum(P * fd for (_, _, fd, tb) in COMP_SEGS if not tb)
N_B = sum(P * fd for (_, _, fd, tb) in COMP_SEGS if tb)

# Exposed for test.py: the BassKernelResults of the last run.
LAST_RESULTS = None
_CACHE = {}


def _ts_rev0(eng, out, in0, s1, s2, op0, op1):
    """tensor_scalar with reverse0: out = (s1 op0 in0) op1 s2.
    Hand-built; the bass Rust wrapper doesn't expose the reverse flags."""
    inst = mybir.InstTensorScalarPtr(
        name=eng.bass.get_next_instruction_name(),
        op0=op0,
        op1=op1,
        reverse0=True,
        ins=[
            eng.lower_ap(in0),
            eng.lower_ap_or_imm(float(s1)),
            eng.lower_ap_or_imm(float(s2)),
        ],
        outs=[eng.lower_ap(out)],
    )
    return eng.add_instruction(inst)


def build_nc():
    nc = bacc.Bacc(dynamic_dma_scratch_size=512)
    x_d = nc.dram_tensor("input", [ROWS, D], F32, kind="ExternalInput")
    t_d = nc.dram_tensor("target", [ROWS, D], F32, kind="ExternalInput")
    qsum_d = nc.dram_tensor("qsum", [1, 2 * MM_N], F32, kind="ExternalOutput")
    acc2_d = nc.dram_tensor("acc2", [P, 2 * N_COLS], F32, kind="ExternalOutput")

    x_t = x_d[:, :].rearrange("(n p) m -> n p m", p=P)
    t_t = t_d[:, :].rearrange("(n p) m -> n p m", p=P)

    add = mybir.AluOpType.add
    sub = mybir.AluOpType.subtract
    amax = mybir.AluOpType.max
    Copy = mybir.ActivationFunctionType.Copy
    Relu = mybir.ActivationFunctionType.Relu
    Abs = mybir.ActivationFunctionType.Abs

    n_mm_a = sum(fd // MM_N for (_, _, fd, tb) in COMP_SEGS if not tb)
    n_mm_b = sum(fd // MM_N for (_, _, fd, tb) in COMP_SEGS if tb)

    with TileContext(nc) as tc:
        with (
            tc.tile_pool(name="iox", bufs=5) as iox_pool,
            tc.tile_pool(name="iot", bufs=3) as iot_pool,
            tc.tile_pool(name="mid", bufs=2) as mid_pool,
            tc.tile_pool(name="fix", bufs=1) as fix_pool,
            tc.tile_pool(name="psum", bufs=1, space="PSUM") as psum_pool,
        ):
            ones = fix_pool.tile([P, 1], BF16)
            bias_nhalf = fix_pool.tile([P, 1], F32)
            nc.vector.memset(ones[:, :], 1.0)
            nc.vector.memset(bias_nhalf[:, :], -0.5)
            qsum = psum_pool.tile([1, MM_N], F32)     # A: sum max(S5,.5)
            qsum2 = psum_pool.tile([1, MM_N], F32)    # B: sum S5
            res = fix_pool.tile([1, 2 * MM_N], F32)
            # acc[:, :N_COLS] = relu sums (all); [:, N_COLS:] = abs (B only)
            acc = fix_pool.tile([P, 2 * N_COLS], F32)

            mm_a = mm_b = 0
            xs = ts = None
            cur_tile = -1
            prev = None  # (S5 tile, col, fd, typeB) pending ACT accums
            for col, (ti, loff, fd, typeB) in enumerate(COMP_SEGS):
                if ti != cur_tile:
                    xs = iox_pool.tile([P, FD], F32, tag="x")
                    ts = iot_pool.tile([P, FD], F32, tag="t")
                    for off, dfd in TILE_DMAS[ti]:
                        nc.sync.dma_start(
                            xs[:, off : off + dfd], x_t[ti][:, off : off + dfd]
                        )
                        nc.sync.dma_start(
                            ts[:, off : off + dfd], t_t[ti][:, off : off + dfd]
                        )
                    cur_tile = ti
                xv = xs[:, loff : loff + fd]
                tv = ts[:, loff : loff + fd]

                y2 = mid_pool.tile([P, HFD], F32, tag="y2")
                x2h = mid_pool.tile([P, HFD], BF16, tag="x2h")
                e = mid_pool.tile([P, HFD], BF16, tag="e")
                S5 = mid_pool.tile([P, HFD], BF16, tag="S5")
                qp = mid_pool.tile([P, HFD], BF16, tag="qp")

                # y2 = (0.5 - x) + M = M - floor(x)  (chain head)
                _ts_rev0(nc.vector, y2[:, :fd], xv, 0.5, MAGIC, sub, add)
                # x2h = 2x - 0.5  (ACT runs one segment ahead of its relu)
                nc.scalar.activation(x2h[:, :fd], xv, Copy, bias=-0.5, scale=2.0)
                # e = (t + M) - y2 = round(t) + floor(x)  (exact, bf16-exact)
                nc.vector.scalar_tensor_tensor(
                    e[:, :fd], tv, MAGIC, y2[:, :fd], add, sub
                )
                # S5 = x2h - e = S - 0.5
                nc.vector.tensor_tensor(S5[:, :fd], x2h[:, :fd], e[:, :fd], sub)
                if typeB:
                    # B plus-branch: PE sums S5 directly (abs comes later
                    # on ACT); no DVE max pass.
                    for k in range(fd // MM_N):
                        nc.tensor.matmul(
                            qsum2[:, :], ones[:, :],
                            S5[:, k * MM_N : (k + 1) * MM_N],
                            start=(mm_b == 0), stop=(mm_b == n_mm_b - 1),
                        )
                        mm_b += 1
                else:
                    # A plus-branch: qp = max(S5, 0.5); PE accumulates
                    nc.vector.tensor_scalar(qp[:, :fd], S5[:, :fd], 0.5, None, amax)
                    for k in range(fd // MM_N):
                        nc.tensor.matmul(
                            qsum[:, :], ones[:, :], qp[:, k * MM_N : (k + 1) * MM_N],
                            start=(mm_a == 0), stop=(mm_a == n_mm_a - 1),
                        )
                        mm_a += 1
                # skewed ACT accums for the previous segment:
                # acc[:,pcol] = sum relu(-S5_prev - 0.5); B also
                # acc[:,N_COLS+pcol] = sum |S5_prev - 0.5|
                if prev is not None:
                    pS5, pcol, pfd, ptb = prev
                    rq = mid_pool.tile([P, HFD], BF16, tag="rq")
                    nc.scalar.activation(
                        rq[:, :pfd], pS5[:, :pfd], Relu,
                        bias=bias_nhalf[:, :], scale=-1.0,
                        accum_out=acc[:, pcol : pcol + 1],
                    )
                    if ptb:
                        rq2 = mid_pool.tile([P, HFD], BF16, tag="rq")
                        nc.scalar.activation(
                            rq2[:, :pfd], pS5[:, :pfd], Abs,
                            bias=bias_nhalf[:, :], scale=1.0,
                            accum_out=acc[:, N_COLS + pcol : N_COLS + pcol + 1],
                        )
                prev = (S5, col, fd, typeB)

            pS5, pcol, pfd, ptb = prev
            rq = mid_pool.tile([P, HFD], BF16, tag="rq")
            nc.scalar.activation(
                rq[:, :pfd], pS5[:, :pfd], Relu,
                bias=bias_nhalf[:, :], scale=-1.0,
                accum_out=acc[:, pcol : pcol + 1],
            )
            if ptb:
                rq2 = mid_pool.tile([P, HFD], BF16, tag="rq")
                nc.scalar.activation(
                    rq2[:, :pfd], pS5[:, :pfd], Abs,
                    bias=bias_nhalf[:, :], scale=1.0,
                    accum_out=acc[:, N_COLS + pcol : N_COLS + pcol + 1],
                )

            nc.scalar.copy(res[:, :MM_N], qsum[:, :])
            nc.scalar.copy(res[:, MM_N:], qsum2[:, :])
            nc.sync.dma_start(qsum_d[:, :], res[:, :])
            nc.sync.dma_start(acc2_d[:, :], acc[:, :])

    nc.compile()
    return nc


def kernel(input, target):
    global LAST_RESULTS
    x = np.ascontiguousarray(np.asarray(input, dtype=np.float32))
    t = np.ascontiguousarray(np.asarray(target, dtype=np.float32))
    assert x.shape == (B, D) and t.shape == (B, D)

    if "nc" not in _CACHE:
        _CACHE["nc"] = build_nc()
    nc = _CACHE["nc"]

    in_maps = []
    for j in range(N_CORES):
        r0, r1 = j * ROWS, (j + 1) * ROWS
        in_maps.append(
            {
                "input": np.ascontiguousarray(x[r0:r1]),
                "target": np.ascontiguousarray(t[r0:r1]),
            }
        )

    res = run_bass_kernel_spmd(nc, in_maps, core_ids=list(range(N_CORES)))
    LAST_RESULTS = res

    b_cols = np.array([tb for (_, _, _, tb) in COMP_SEGS], dtype=bool)
    q_a = q_b = s2 = s3 = 0.0
    for j in range(N_CORES):
        q = res.results[j]["qsum"].astype(np.float64)
        q_a += q[0, :MM_N].sum()
        q_b += q[0, MM_N:].sum()
        a = res.results[j]["acc2"].astype(np.float64)
        s2 += a[:, :N_COLS].sum()                 # relu(-S5-.5), all segs
        s3 += a[:, N_COLS:][:, b_cols].sum()      # |S5-.5|, B segs only

    # sum(loss) = sum relu(S5-.5) + sum relu(-S5-.5)
    #   A segs: relu+ = sum max(S5,.5) - N_A/2          (q_a)
    #   B segs: relu+ = (sum_B S5)/2 - N_B/4 + (sum_B |S5-.5|)/2
    n = float(B) * float(D)
    loss_sum = (
        q_a - N_CORES * N_A / 2.0
        + 0.5 * q_b - N_CORES * N_B / 4.0 + 0.5 * s3
        + s2
    )
    return np.float32(loss_sum / n)


# revision 12
# speedup vs baseline: 1.0344x; 1.0344x over previous
"""Distributed Trainium2 kernel for nn_ARLoss_88390426951926.

Computes mean(loss) where, per element (EPS = 1e-6):
    c = round(t); d = x - c; pos = d >= 0
    z = pos ? ceil(x) : floor(x)
    loss = max(0, |d| - |x - z| + pos*EPS)

Algebraic reduction (validated ~5e-6 rel err on the real data):
    With F = floor(x), S = 2x - c - F, S5 = S - 0.5:
        loss = relu(S - 1 + eps) + relu(-S)         (ties measure-zero)
so  sum(loss) = sum relu(S5 - .5) + sum relu(-S5 - .5)
             = sum max(S5, .5) - N/2 + sum relu(-S5 - .5).

Rounding via the float32 magic trick (M = 1.5*2^23), fused to minimize
DVE passes (reverse0 on tensor_scalar + scalar_tensor_tensor both
HW-verified exact by probe.py):
    y2 = (0.5 - x) + M     = M - floor(x)        DVE TS2op reverse0, f32
    e  = (t + M) - y2      = round(t) + floor(x) DVE STT, f32 chain->bf16
    x2h = Copy(2x - 0.5)                         ACT, f32->bf16
    S5 = x2h - e                                 DVE TT, bf16
    qp = max(S5, 0.5); PE psum[1,512] += ones.T @ qp
    ACT relu-accum: acc2[:,col] = sum relu(-S5 - 0.5)   (all segments)
The relu-accum for segment k is emitted during segment k+1 so ACT's
x2h pass runs one segment ahead and never stalls the DVE chain.
Per-core budget (measured): DVE ~72us, ACT ~72us, PE ~45us under the
~94-102us HBM DMA floor (33.5 MB @ ~330-358 GB/s).
First and last tiles are split into 4 quarter tiles so the pipeline
ramps in and drains out quickly.
"""

import sys
import types

import numpy as np

import concourse.bass as bass
import concourse.bacc as bacc
import concourse.mybir as mybir
from concourse.tile import TileContext
from concourse.bass_utils import run_bass_kernel_spmd


def _ensure_axon_hooks():
    """Some agent images lack ``antenv.axon_hooks``; if BASS_TRACE is set
    in the environment, run_bass_kernel_spmd imports it and would crash.
    Provide a no-op hook registry so tracing degrades gracefully."""
    try:
        import antenv  # noqa: F401
    except ImportError:
        return
    try:
        import antenv.axon_hooks  # noqa: F401
        return
    except ImportError:
        pass
    mod = types.ModuleType("antenv.axon_hooks")
    _state = {"hook": None}
    mod.set_axon_ntff_profile_hook = lambda h: _state.__setitem__("hook", h)
    mod.get_axon_ntff_profile_hook = lambda: _state["hook"]
    sys.modules["antenv.axon_hooks"] = mod
    import antenv as _a

    _a.axon_hooks = mod


_ensure_axon_hooks()

B, D = 8192, 4096
N_CORES = 8
ROWS = B // N_CORES          # 1024 rows per core
P = 128                      # SBUF partitions
FD = 4096                    # free dim per full tile
NTILES = (ROWS * D) // (P * FD)   # 8 full tiles per core
MAGIC = 12582912.0           # 1.5 * 2**23
MM_N = 512                   # matmul free-dim chunk (one PSUM bank)

F32 = mybir.dt.float32
BF16 = mybir.dt.bfloat16

HFD = FD // 2                # compute half-tile width (smaller mid pool)

# Per-tile DMA split: first/last tiles in graduated sub-transfers (fast
# ramp/drain) but into ONE SBUF slot each, so the splits don't burn
# extra tile-pool slots; middle tiles as full 2 MiB transfers.
TILE_DMAS = []
for _ti in range(NTILES):
    if _ti == 0:
        TILE_DMAS.append([(0, 512), (512, 512), (1024, 1024), (2048, 2048)])
    elif _ti == NTILES - 1:
        TILE_DMAS.append([(0, 2048), (2048, 1024), (3072, 512), (3584, 512)])
    else:
        TILE_DMAS.append([(0, FD)])

# Compute segments: (tile_idx, tile_off, fd, typeB). Edge tiles are
# computed in segments aligned with their sub-DMAs; full tiles in two
# half-tiles so mid intermediates are [P, HFD]. typeB segments compute
# the plus-branch as 1/2*(PE sum(S5) + ACT sum|S5-.5|) instead of the
# DVE max pass, moving ~690ns/half from the DVE (the binding engine)
# to ACT+PE slack. Alternating halves in the middle tiles gives an
# element fraction ~0.375, near the DVE==ACT balance point.
COMP_SEGS = []
_hidx = 0
for _ti, _subs in enumerate(TILE_DMAS):
    for _off, _dfd in _subs:
        for _h in range(max(1, _dfd // HFD)):
            _fd = min(HFD, _dfd)
            _tb = False
            if _fd == HFD and 0 < _ti < NTILES - 1:
                _tb = _hidx in (1, 3, 5, 7, 9)
                _hidx += 1
            COMP_SEGS.append((_ti, _off + _h * HFD, _fd, _tb))
N_COLS = len(COMP_SEGS)
N_A = sum(P * fd for (_, _, fd, tb) in COMP_SEGS if not tb)
N_B = sum(P * fd for (_, _, fd, tb) in COMP_SEGS if tb)

# Exposed for test.py: the BassKernelResults of the last run.
LAST_RESULTS = None
_CACHE = {}


def _ts_rev0(eng, out, in0, s1, s2, op0, op1):
    """tensor_scalar with reverse0: out = (s1 op0 in0) op1 s2.
    Hand-built; the bass Rust wrapper doesn't expose the reverse flags."""
    inst = mybir.InstTensorScalarPtr(
        name=eng.bass.get_next_instruction_name(),
        op0=op0,
        op1=op1,
        reverse0=True,
        ins=[
            eng.lower_ap(in0),
            eng.lower_ap_or_imm(float(s1)),
            eng.lower_ap_or_imm(float(s2)),
        ],
        outs=[eng.lower_ap(out)],
    )
    return eng.add_instruction(inst)


def build_nc():
    nc = bacc.Bacc(dynamic_dma_scratch_size=512)
    x_d = nc.dram_tensor("input", [ROWS, D], F32, kind="ExternalInput")
    t_d = nc.dram_tensor("target", [ROWS, D], F32, kind="ExternalInput")
    qsum_d = nc.dram_tensor("qsum", [1, 2 * MM_N], F32, kind="ExternalOutput")
    acc2_d = nc.dram_tensor("acc2", [P, 2 * N_COLS], F32, kind="ExternalOutput")

    x_t = x_d[:, :].rearrange("(n p) m -> n p m", p=P)
    t_t = t_d[:, :].rearrange("(n p) m -> n p m", p=P)

    add = mybir.AluOpType.add
    sub = mybir.AluOpType.subtract
    amax = mybir.AluOpType.max
    Copy = mybir.ActivationFunctionType.Copy
    Relu = mybir.ActivationFunctionType.Relu
    Abs = mybir.ActivationFunctionType.Abs

    n_mm_a = sum(fd // MM_N for (_, _, fd, tb) in COMP_SEGS if not tb)
    n_mm_b = sum(fd // MM_N for (_, _, fd, tb) in COMP_SEGS if tb)

    with TileContext(nc) as tc:
        with (
            tc.tile_pool(name="iox", bufs=5) as iox_pool,
            tc.tile_pool(name="iot", bufs=3) as iot_pool,
            tc.tile_pool(name="mid", bufs=2) as mid_pool,
            tc.tile_pool(name="s5p", bufs=3) as s5_pool,
            tc.tile_pool(name="fix", bufs=1) as fix_pool,
            tc.tile_pool(name="psum", bufs=1, space="PSUM") as psum_pool,
        ):
            ones = fix_pool.tile([P, 1], BF16)
            bias_nhalf = fix_pool.tile([P, 1], F32)
            nc.vector.memset(ones[:, :], 1.0)
            nc.vector.memset(bias_nhalf[:, :], -0.5)
            qsum = psum_pool.tile([1, MM_N], F32)     # A: sum max(S5,.5)
            qsum2 = psum_pool.tile([1, MM_N], F32)    # B: sum S5
            res = fix_pool.tile([1, 2 * MM_N], F32)
            # acc[:, :N_COLS] = relu sums (all); [:, N_COLS:] = abs (B only)
            acc = fix_pool.tile([P, 2 * N_COLS], F32)

            mm_a = mm_b = 0
            xs = ts = None
            cur_tile = -1
            prev = None  # (S5 tile, col, fd, typeB) pending ACT accums
            for col, (ti, loff, fd, typeB) in enumerate(COMP_SEGS):
                if ti != cur_tile:
                    xs = iox_pool.tile([P, FD], F32, tag="x")
                    ts = iot_pool.tile([P, FD], F32, tag="t")
                    for off, dfd in TILE_DMAS[ti]:
                        nc.sync.dma_start(
                            xs[:, off : off + dfd], x_t[ti][:, off : off + dfd]
                        )
                        nc.sync.dma_start(
                            ts[:, off : off + dfd], t_t[ti][:, off : off + dfd]
                        )
                    cur_tile = ti
                xv = xs[:, loff : loff + fd]
                tv = ts[:, loff : loff + fd]

                y2 = mid_pool.tile([P, HFD], F32, tag="y2")
                x2h = mid_pool.tile([P, HFD], BF16, tag="x2h")
                e = mid_pool.tile([P, HFD], BF16, tag="e")
                S5 = s5_pool.tile([P, HFD], BF16, tag="S5")
                qp = mid_pool.tile([P, HFD], BF16, tag="qp")

                # y2 = (0.5 - x) + M = M - floor(x)  (chain head)
                _ts_rev0(nc.vector, y2[:, :fd], xv, 0.5, MAGIC, sub, add)
                # x2h = 2x - 0.5  (ACT runs one segment ahead of its relu)
                nc.scalar.activation(x2h[:, :fd], xv, Copy, bias=-0.5, scale=2.0)
                # e = (t + M) - y2 = round(t) + floor(x)  (exact, bf16-exact)
                nc.vector.scalar_tensor_tensor(
                    e[:, :fd], tv, MAGIC, y2[:, :fd], add, sub
                )
                # S5 = x2h - e = S - 0.5
                nc.vector.tensor_tensor(S5[:, :fd], x2h[:, :fd], e[:, :fd], sub)
                if typeB:
                    # B plus-branch: PE sums S5 directly (abs comes later
                    # on ACT); no DVE max pass.
                    for k in range(fd // MM_N):
                        nc.tensor.matmul(
                            qsum2[:, :], ones[:, :],
                            S5[:, k * MM_N : (k + 1) * MM_N],
                            start=(mm_b == 0), stop=(mm_b == n_mm_b - 1),
                        )
                        mm_b += 1
                else:
                    # A plus-branch: qp = max(S5, 0.5); PE accumulates
                    nc.vector.tensor_scalar(qp[:, :fd], S5[:, :fd], 0.5, None, amax)
                    for k in range(fd // MM_N):
                        nc.tensor.matmul(
                            qsum[:, :], ones[:, :], qp[:, k * MM_N : (k + 1) * MM_N],
                            start=(mm_a == 0), stop=(mm_a == n_mm_a - 1),
                        )
                        mm_a += 1
                # skewed ACT accums for the previous segment:
                # acc[:,pcol] = sum relu(-S5_prev - 0.5); B also
                # acc[:,N_COLS+pcol] = sum |S5_prev - 0.5|
                if prev is not None:
                    pS5, pcol, pfd, ptb = prev
                    rq = mid_pool.tile([P, HFD], BF16, tag="rq")
                    nc.scalar.activation(
                        rq[:, :pfd], pS5[:, :pfd], Relu,
                        bias=bias_nhalf[:, :], scale=-1.0,
                        accum_out=acc[:, pcol : pcol + 1],
                    )
                    if ptb:
                        rq2 = mid_pool.tile([P, HFD], BF16, tag="rq")
                        nc.scalar.activation(
                            rq2[:, :pfd], pS5[:, :pfd], Abs,
                            bias=bias_nhalf[:, :], scale=1.0,
                            accum_out=acc[:, N_COLS + pcol : N_COLS + pcol + 1],
                        )
                prev = (S5, col, fd, typeB)

            pS5, pcol, pfd, ptb = prev
            rq = mid_pool.tile([P, HFD], BF16, tag="rq")
            nc.scalar.activation(
                rq[:, :pfd], pS5[:, :pfd], Relu,
                bias=bias_nhalf[:, :], scale=-1.0,
                accum_out=acc[:, pcol : pcol + 1],
            )
            if ptb:
                rq2 = mid_pool.tile([P, HFD], BF16, tag="rq")
                nc.scalar.activation(
                    rq2[:, :pfd], pS5[:, :pfd], Abs,
                    bias=bias_nhalf[:, :], scale=1.0,
                    accum_out=acc[:, N_COLS + pcol : N_COLS + pcol + 1],
                )

            nc.vector.tensor_scalar(res[:, :MM_N], qsum[:, :], 0.0, None, add)
            nc.vector.tensor_scalar(res[:, MM_N:], qsum2[:, :], 0.0, None, add)
            nc.sync.dma_start(qsum_d[:, :], res[:, :])
            nc.sync.dma_start(acc2_d[:, :], acc[:, :])

    nc.compile()
    return nc


def kernel(input, target):
    global LAST_RESULTS
    x = np.ascontiguousarray(np.asarray(input, dtype=np.float32))
    t = np.ascontiguousarray(np.asarray(target, dtype=np.float32))
    assert x.shape == (B, D) and t.shape == (B, D)

    if "nc" not in _CACHE:
        _CACHE["nc"] = build_nc()
    nc = _CACHE["nc"]

    in_maps = []
    for j in range(N_CORES):
        r0, r1 = j * ROWS, (j + 1) * ROWS
        in_maps.append(
            {
                "input": np.ascontiguousarray(x[r0:r1]),
                "target": np.ascontiguousarray(t[r0:r1]),
            }
        )

    res = run_bass_kernel_spmd(nc, in_maps, core_ids=list(range(N_CORES)))
    LAST_RESULTS = res

    b_cols = np.array([tb for (_, _, _, tb) in COMP_SEGS], dtype=bool)
    q_a = q_b = s2 = s3 = 0.0
    for j in range(N_CORES):
        q = res.results[j]["qsum"].astype(np.float64)
        q_a += q[0, :MM_N].sum()
        q_b += q[0, MM_N:].sum()
        a = res.results[j]["acc2"].astype(np.float64)
        s2 += a[:, :N_COLS].sum()                 # relu(-S5-.5), all segs
        s3 += a[:, N_COLS:][:, b_cols].sum()      # |S5-.5|, B segs only

    # sum(loss) = sum relu(S5-.5) + sum relu(-S5-.5)
    #   A segs: relu+ = sum max(S5,.5) - N_A/2          (q_a)
    #   B segs: relu+ = (sum_B S5)/2 - N_B/4 + (sum_B |S5-.5|)/2
    n = float(B) * float(D)
    loss_sum = (
        q_a - N_CORES * N_A / 2.0
        + 0.5 * q_b - N_CORES * N_B / 4.0 + 0.5 * s3
        + s2
    )
    return np.float32(loss_sum / n)
